# revision 10
# baseline (speedup 1.0000x reference)
"""BERT self-attention on 8 Trainium2 NeuronCores.

Reference computes, for x [2, 2048, 1024] and 16 heads of dim 64:
    q,k,v = (x@w* + b*) split into heads
    attn  = softmax(q k^T / 8)              -> [2, 16, 2048, 2048] (output 2)
    out   = (attn v) @ wo + bo              -> [2, 2048, 1024]     (output 1)

Sharding: core c handles batch c//4 and heads 4*(c%4) .. +4 (tensor parallel
over heads, data parallel over batch). Each core computes a full partial of
the output projection over its 4 heads; host sums the 4 partials per batch
(cheaper than an on-device all-reduce) and adds bo.

Per-core kernel (all matmuls fp32r except attn*V in bf16):
  A: QT/KT [head_dim*2, s] per head-pair and V4 [t, 256] projections from a
     host-pre-transposed xT, biases fused into the PSUM->SBUF copies.
  B: per s-tile: scores for both heads of a pair concurrently (K=64
     row-packing via tile_position), ACT exp with accum_out row sums,
     DVE reciprocal + normalize, DMA of normalized attn rows.
  C: per t-tile: transposed scores, exp -> bf16, bf16 attn*V accumulation
     (M=64 col-packing); then denominators broadcast across partitions via
     K=1 ones-matmuls and DVE normalize of the [head_dim, s] result.
  D: output projection (K=256 over the two pair chunks) -> partial out.
"""

from contextlib import ExitStack

import numpy as np

import concourse.mybir as mybir
import concourse.tile as tile
from concourse import bacc
from concourse.bass_utils import run_bass_kernel_spmd
from concourse.masks import make_identity

F32 = mybir.dt.float32
F32R = mybir.dt.float32r
BF16 = mybir.dt.bfloat16
AF = mybir.ActivationFunctionType

N_CORES = 8
B = 2
S = 2048
D = 1024
H = 16
DK = 64
HEADS_PER_CORE = 4  # 2 pairs
PAIRS = 2
ST = S // 128  # 16 s/t tiles
KC = D // 128  # 8 contraction chunks
SG = S // 512  # 4 column groups of 512

_CACHE = {}


def build():
    nc = bacc.Bacc("TRN2", target_bir_lowering=False, debug=False)
    xt = nc.dram_tensor("xt", [D, S], F32, kind="ExternalInput").ap()
    wq = nc.dram_tensor("wq", [D, 256], F32, kind="ExternalInput").ap()
    wk = nc.dram_tensor("wk", [D, 256], F32, kind="ExternalInput").ap()
    wv = nc.dram_tensor("wv", [D, 256], F32, kind="ExternalInput").ap()
    wo = nc.dram_tensor("wo", [256, D], F32, kind="ExternalInput").ap()
    bq = nc.dram_tensor("bq", [256], F32, kind="ExternalInput").ap()
    bk = nc.dram_tensor("bk", [256], F32, kind="ExternalInput").ap()
    bv = nc.dram_tensor("bv", [256], F32, kind="ExternalInput").ap()
    attn = nc.dram_tensor("attn", [HEADS_PER_CORE, S, S], F32, kind="ExternalOutput").ap()
    outp = nc.dram_tensor("outp", [S, D], F32, kind="ExternalOutput").ap()

    with tile.TileContext(nc) as tc, ExitStack() as ctx:
        # long-lived tiles
        persist = ctx.enter_context(tc.tile_pool(name="persist", bufs=1))

        qt_sb = persist.tile([128, PAIRS, S], F32R, tag="qt")  # [hd, pair, s]
        kt_sb = persist.tile([128, PAIRS, S], F32R, tag="kt")
        v4_sb = persist.tile([128, ST, 256], BF16, tag="v4")  # [t, t_tile, hd]
        avt_sb = persist.tile([128, PAIRS, S], F32R, tag="avt")  # normalized (A@V)^T
        wo_sb = persist.tile([128, PAIRS, D], F32R, tag="wo")
        rrec_cols = persist.tile([128, HEADS_PER_CORE, ST], F32, tag="rrec_cols")
        rrec_rows = persist.tile([1, HEADS_PER_CORE, S], F32R, tag="rrec_rows")
        ones1_f32 = persist.tile([1, 128], F32, tag="ones1_f32")
        ones1 = persist.tile([1, 128], F32R, tag="ones1")
        ident = persist.tile([128, 128], F32, tag="ident")
        bqk_sb = persist.tile([128, 2, PAIRS], F32, tag="bqk")  # [p, (bq|bk), pair]
        bv_row = persist.tile([1, 256], F32R, tag="bv_row")
        bv_bc = persist.tile([128, 256], F32, tag="bv_bc")

        nc.gpsimd.memset(ones1_f32[:], 1.0)
        nc.vector.tensor_copy(ones1[:], ones1_f32[:])
        make_identity(nc, ident[:])
        # gpsimd (SWDGE) DMA casts f32 -> f32r (rounding the producer, which
        # the BIR verifier requires for fp32r matmul operands)
        nc.gpsimd.dma_start(wo_sb[:], wo.rearrange("(c p) d -> p c d", p=128))
        nc.sync.dma_start(bqk_sb[:, 0, :], bq.rearrange("(c p) -> p c", p=128))
        nc.sync.dma_start(bqk_sb[:, 1, :], bk.rearrange("(c p) -> p c", p=128))
        nc.gpsimd.dma_start(bv_row[:], bv.rearrange("(o d) -> o d", o=1))

        with (
            tc.tile_pool(name="proj_in", bufs=1) as proj_in,
            tc.tile_pool(name="proj_ps", bufs=2, space="PSUM") as proj_ps,
        ):
            xt_sb = proj_in.tile([128, KC, S], F32R, tag="xt")
            wq_sb = proj_in.tile([128, KC, 256], F32R, tag="wq")
            wk_sb = proj_in.tile([128, KC, 256], F32R, tag="wk")
            wv_sb = proj_in.tile([128, KC, 256], F32R, tag="wv")
            nc.gpsimd.dma_start(xt_sb[:], xt.rearrange("(c p) s -> p c s", p=128))
            nc.gpsimd.dma_start(wq_sb[:], wq.rearrange("(c p) d -> p c d", p=128))
            nc.gpsimd.dma_start(wk_sb[:], wk.rearrange("(c p) d -> p c d", p=128))
            nc.gpsimd.dma_start(wv_sb[:], wv.rearrange("(c p) d -> p c d", p=128))

            # bv broadcast to 128 partitions (K=1 ones matmul)
            bv_ps = proj_ps.tile([128, 256], F32, tag="bv_ps")
            nc.tensor.matmul(bv_ps[:], ones1[:], bv_row[:], start=True, stop=True)
            nc.vector.tensor_copy(bv_bc[:], bv_ps[:])

            # Q/K projections: QT[hd, s] = wq_slice^T @ xT per pair
            for w_sb, dst, bcol in ((wq_sb, qt_sb, 0), (wk_sb, kt_sb, 1)):
                for p2 in range(PAIRS):
                    for g in range(SG):
                        qk_ps = proj_ps.tile([128, 512], F32, tag="qk_ps")
                        for k in range(KC):
                            nc.tensor.matmul(
                                qk_ps[:],
                                w_sb[:, k, 128 * p2 : 128 * (p2 + 1)],
                                xt_sb[:, k, 512 * g : 512 * (g + 1)],
                                start=(k == 0),
                                stop=(k == KC - 1),
                            )
                        nc.vector.tensor_scalar_add(
                            dst[:, p2, 512 * g : 512 * (g + 1)],
                            qk_ps[:],
                            bqk_sb[:, bcol, p2 : p2 + 1],
                        )

            # V projection directly in [t, hd] layout: V4 = xT^T-chunk @ wv
            for t in range(ST):
                v_ps = proj_ps.tile([128, 256], F32, tag="v_ps")
                for k in range(KC):
                    nc.tensor.matmul(
                        v_ps[:],
                        xt_sb[:, k, 128 * t : 128 * (t + 1)],
                        wv_sb[:, k, :],
                        start=(k == 0),
                        stop=(k == KC - 1),
                    )
                nc.vector.tensor_add(v4_sb[:, t, :], v_ps[:], bv_bc[:])

        for p2 in range(PAIRS):
            # ---- phase B: scores [s, t], softmax, attn output ----
            with (
                tc.tile_pool(name="sc_ps", bufs=2, space="PSUM") as sc_pool,
                tc.tile_pool(name="b_sb", bufs=3) as b_sb,
                tc.tile_pool(name="bn_sb", bufs=6) as bn_sb,
            ):
                for st in range(ST):
                    for h in range(2):
                        hh = 2 * p2 + h
                        sc_ps = sc_pool.tile([128, S], F32, tag="sc")
                        for g in range(SG):
                            nc.tensor.matmul(
                                sc_ps[:, 512 * g : 512 * (g + 1)],
                                qt_sb[64 * h : 64 * (h + 1), p2, 128 * st : 128 * (st + 1)],
                                kt_sb[64 * h : 64 * (h + 1), p2, 512 * g : 512 * (g + 1)],
                                start=True,
                                stop=True,
                                tile_position=(64 * h, 0),
                            )
                        a_sb = b_sb.tile([128, S], F32, tag="a")
                        rsum = b_sb.tile([128, 1], F32, tag="rsum")
                        nc.scalar.activation(
                            a_sb[:], sc_ps[:], AF.Exp, scale=0.125, accum_out=rsum[:]
                        )
                        nc.vector.reciprocal(rrec_cols[:, hh, st : st + 1], rsum[:])
                        an_sb = bn_sb.tile([128, S], F32, tag="an")
                        nc.vector.tensor_scalar_mul(
                            an_sb[:], a_sb[:], rrec_cols[:, hh, st : st + 1]
                        )
                        nc.sync.dma_start(
                            attn[hh, 128 * st : 128 * (st + 1), :], an_sb[:]
                        )

            # denominator reciprocals, transposed to a single-partition row
            # layout (matmul rhs base partition must be 0)
            with tc.tile_pool(name="tr_ps", bufs=2, space="PSUM") as tr_pool:
                for h in range(2):
                    hh = 2 * p2 + h
                    for st in range(ST):
                        rr_ps = tr_pool.tile([1, 128], F32, tag="rr")
                        nc.tensor.transpose(
                            rr_ps[:], rrec_cols[:, hh, st : st + 1], ident[:]
                        )
                        nc.vector.tensor_copy(
                            rrec_rows[:, hh, 128 * st : 128 * (st + 1)], rr_ps[:]
                        )

            # ---- phase C: transposed scores, exp -> bf16, attn*V ----
            with (
                tc.tile_pool(name="av_ps", bufs=1, space="PSUM") as av_pool,
                tc.tile_pool(name="c_sb", bufs=3) as c_sb,
                tc.tile_pool(name="sct_ps", bufs=1, space="PSUM") as sct_pool,
            ):
                av_ps = av_pool.tile([128, S], F32, tag="av")
                for tt in range(ST):
                    for h in range(2):
                        sct_ps = sct_pool.tile([128, S], F32, tag="sct")
                        for g in range(SG):
                            nc.tensor.matmul(
                                sct_ps[:, 512 * g : 512 * (g + 1)],
                                kt_sb[64 * h : 64 * (h + 1), p2, 128 * tt : 128 * (tt + 1)],
                                qt_sb[64 * h : 64 * (h + 1), p2, 512 * g : 512 * (g + 1)],
                                start=True,
                                stop=True,
                                tile_position=(64 * h, 0),
                            )
                        at_sb = c_sb.tile([128, S], BF16, tag="at")
                        nc.scalar.activation(at_sb[:], sct_ps[:], AF.Exp, scale=0.125)
                        hh = 2 * p2 + h
                        for g in range(SG):
                            nc.tensor.matmul(
                                av_ps[64 * h : 64 * (h + 1), 512 * g : 512 * (g + 1)],
                                v4_sb[:, tt, 64 * hh : 64 * (hh + 1)],
                                at_sb[:, 512 * g : 512 * (g + 1)],
                                start=(tt == 0),
                                stop=(tt == ST - 1),
                                tile_position=(0, 64 * h),
                            )

                # normalize AVT columns by the per-(s, head) reciprocal sums
                # (reuse the sct psum slot, free by now, via the same pool/tag)
                if True:
                    r2_pool = sct_pool
                    r2_ps = r2_pool.tile([128, S], F32, tag="sct")
                    for h in range(2):
                        hh = 2 * p2 + h
                        for st in range(ST):
                            nc.tensor.matmul(
                                r2_ps[:, 128 * st : 128 * (st + 1)],
                                ones1[:],
                                rrec_rows[:, hh, 128 * st : 128 * (st + 1)],
                                start=True,
                                stop=True,
                            )
                        r2_sb = c_sb.tile([128, S], F32, tag="r2sb")
                        nc.vector.tensor_copy(
                            r2_sb[64 * h : 64 * (h + 1), :],
                            r2_ps[64 * h : 64 * (h + 1), :],
                        )
                        nc.vector.tensor_mul(
                            avt_sb[64 * h : 64 * (h + 1), p2, :],
                            av_ps[64 * h : 64 * (h + 1), :],
                            r2_sb[64 * h : 64 * (h + 1), :],
                        )

        # ---- phase D: output projection, partial over this core's heads ----
        with (
            tc.tile_pool(name="o_ps", bufs=4, space="PSUM") as o_pool,
            tc.tile_pool(name="o_sb", bufs=3) as o_sb,
        ):
            for st in range(ST):
                out_sb = o_sb.tile([128, D], F32, tag="out")
                for g in range(2):
                    o_ps = o_pool.tile([128, 512], F32, tag="o")
                    for p2 in range(PAIRS):
                        nc.tensor.matmul(
                            o_ps[:],
                            avt_sb[:, p2, 128 * st : 128 * (st + 1)],
                            wo_sb[:, p2, 512 * g : 512 * (g + 1)],
                            start=(p2 == 0),
                            stop=(p2 == PAIRS - 1),
                        )
                    nc.vector.tensor_copy(out_sb[:, 512 * g : 512 * (g + 1)], o_ps[:])
                nc.sync.dma_start(outp[128 * st : 128 * (st + 1), :], out_sb[:])

    nc.compile()
    return nc


def get_nc():
    if "nc" not in _CACHE:
        _CACHE["nc"] = build()
    return _CACHE["nc"]


def kernel(x, wq, bq, wk, bk, wv, bv, wo, bo):
    x = np.asarray(x, dtype=np.float32)
    wq, bq = np.asarray(wq, np.float32), np.asarray(bq, np.float32)
    wk, bk = np.asarray(wk, np.float32), np.asarray(bk, np.float32)
    wv, bv = np.asarray(wv, np.float32), np.asarray(bv, np.float32)
    wo, bo = np.asarray(wo, np.float32), np.asarray(bo, np.float32)

    nc = get_nc()

    xts = [np.ascontiguousarray(x[b].T) for b in range(B)]
    in_maps = []
    for c in range(N_CORES):
        b, hq = c // 4, c % 4
        lo = 256 * hq
        in_maps.append(
            {
                "xt": xts[b],
                "wq": np.ascontiguousarray(wq[:, lo : lo + 256]),
                "wk": np.ascontiguousarray(wk[:, lo : lo + 256]),
                "wv": np.ascontiguousarray(wv[:, lo : lo + 256]),
                "wo": np.ascontiguousarray(wo[lo : lo + 256, :]),
                "bq": np.ascontiguousarray(bq[lo : lo + 256]),
                "bk": np.ascontiguousarray(bk[lo : lo + 256]),
                "bv": np.ascontiguousarray(bv[lo : lo + 256]),
            }
        )

    r = run_bass_kernel_spmd(nc, in_maps, list(range(N_CORES)))

    attn = np.empty((B, H, S, S), dtype=np.float32)
    out = np.zeros((B, S, D), dtype=np.float32)
    for c in range(N_CORES):
        b, hq = c // 4, c % 4
        attn[b, 4 * hq : 4 * hq + 4] = r.results[c]["attn"]
        out[b] += r.results[c]["outp"]
    out += bo
    return out, attn


# revision 11
# speedup vs baseline: 1.2417x; 1.2417x over previous
"""BERT self-attention on 8 Trainium2 NeuronCores.

Reference computes, for x [2, 2048, 1024] and 16 heads of dim 64:
    q,k,v = (x@w* + b*) split into heads
    attn  = softmax(q k^T / 8)              -> [2, 16, 2048, 2048] (output 2)
    out   = (attn v) @ wo + bo              -> [2, 2048, 1024]     (output 1)

Sharding: core c handles batch c//4 and heads 4*(c%4) .. +4 (tensor parallel
over heads, data parallel over batch). Each core computes a full partial of
the output projection over its 4 heads; host sums the 4 partials per batch
(cheaper than an on-device all-reduce) and adds bo.

Per-core kernel (all matmuls fp32r except attn*V in bf16):
  A: QT/KT [head_dim*2, s] per head-pair and V4 [t, 256] projections from a
     host-pre-transposed xT, biases fused into the PSUM->SBUF copies.
  B: per s-tile: scores for both heads of a pair concurrently (K=64
     row-packing via tile_position), ACT exp with accum_out row sums,
     DVE reciprocal + normalize, DMA of normalized attn rows.
  C: per t-tile: transposed scores, exp -> bf16, bf16 attn*V accumulation
     (M=64 col-packing); then denominators broadcast across partitions via
     K=1 ones-matmuls and DVE normalize of the [head_dim, s] result.
  D: output projection (K=256 over the two pair chunks) -> partial out.
"""

from contextlib import ExitStack

import numpy as np

import concourse.mybir as mybir
import concourse.tile as tile
from concourse import bacc
from concourse.bass_utils import run_bass_kernel_spmd
from concourse.masks import make_identity

F32 = mybir.dt.float32
F32R = mybir.dt.float32r
BF16 = mybir.dt.bfloat16
AF = mybir.ActivationFunctionType

N_CORES = 8
B = 2
S = 2048
D = 1024
H = 16
DK = 64
HEADS_PER_CORE = 4  # 2 pairs
PAIRS = 2
ST = S // 128  # 16 s/t tiles
KC = D // 128  # 8 contraction chunks
SG = S // 512  # 4 column groups of 512

_CACHE = {}


def build():
    nc = bacc.Bacc("TRN2", target_bir_lowering=False, debug=False)
    xt = nc.dram_tensor("xt", [D, S], F32R, kind="ExternalInput").ap()
    wq = nc.dram_tensor("wq", [D, 256], F32R, kind="ExternalInput").ap()
    wk = nc.dram_tensor("wk", [D, 256], F32R, kind="ExternalInput").ap()
    wv = nc.dram_tensor("wv", [D, 256], F32R, kind="ExternalInput").ap()
    wo = nc.dram_tensor("wo", [256, D], F32R, kind="ExternalInput").ap()
    bq = nc.dram_tensor("bq", [256], F32, kind="ExternalInput").ap()
    bk = nc.dram_tensor("bk", [256], F32, kind="ExternalInput").ap()
    bv = nc.dram_tensor("bv", [256], F32R, kind="ExternalInput").ap()
    attn = nc.dram_tensor("attn", [HEADS_PER_CORE, S, S], F32, kind="ExternalOutput").ap()
    outp = nc.dram_tensor("outp", [S, D], F32, kind="ExternalOutput").ap()

    with tile.TileContext(nc) as tc, ExitStack() as ctx:
        # long-lived tiles
        persist = ctx.enter_context(tc.tile_pool(name="persist", bufs=1))

        qt_sb = persist.tile([128, PAIRS, S], F32R, tag="qt")  # [hd, pair, s]
        kt_sb = persist.tile([128, PAIRS, S], F32R, tag="kt")
        v4_sb = persist.tile([128, ST, 256], BF16, tag="v4")  # [t, t_tile, hd]
        avt_sb = persist.tile([128, PAIRS, S], F32R, tag="avt")  # normalized (A@V)^T
        wo_sb = persist.tile([128, PAIRS, D], F32R, tag="wo")
        rrec_cols = persist.tile([128, HEADS_PER_CORE, ST], F32, tag="rrec_cols")
        rrec_rows = persist.tile([1, HEADS_PER_CORE, S], F32R, tag="rrec_rows")
        ones1_f32 = persist.tile([1, 128], F32, tag="ones1_f32")
        ones1 = persist.tile([1, 128], F32R, tag="ones1")
        ident = persist.tile([128, 128], F32, tag="ident")
        bqk_sb = persist.tile([128, 2, PAIRS], F32, tag="bqk")  # [p, (bq|bk), pair]
        bv_row = persist.tile([1, 256], F32R, tag="bv_row")
        bv_bc = persist.tile([128, 256], F32, tag="bv_bc")

        nc.gpsimd.memset(ones1_f32[:], 1.0)
        nc.vector.tensor_copy(ones1[:], ones1_f32[:])
        make_identity(nc, ident[:])
        # matmul inputs are declared float32r in DRAM (same bits as f32), so
        # plain HWDGE DMA satisfies the verifier's fp32r producer rule
        nc.sync.dma_start(wo_sb[:], wo.rearrange("(c p) d -> p c d", p=128))
        nc.sync.dma_start(bqk_sb[:, 0, :], bq.rearrange("(c p) -> p c", p=128))
        nc.sync.dma_start(bqk_sb[:, 1, :], bk.rearrange("(c p) -> p c", p=128))
        nc.sync.dma_start(bv_row[:], bv.rearrange("(o d) -> o d", o=1))

        with (
            tc.tile_pool(name="proj_in", bufs=1) as proj_in,
            tc.tile_pool(name="proj_ps", bufs=2, space="PSUM") as proj_ps,
        ):
            xt_sb = proj_in.tile([128, KC, S], F32R, tag="xt")
            wq_sb = proj_in.tile([128, KC, 256], F32R, tag="wq")
            wk_sb = proj_in.tile([128, KC, 256], F32R, tag="wk")
            wv_sb = proj_in.tile([128, KC, 256], F32R, tag="wv")
            nc.sync.dma_start(xt_sb[:], xt.rearrange("(c p) s -> p c s", p=128))
            nc.sync.dma_start(wq_sb[:], wq.rearrange("(c p) d -> p c d", p=128))
            nc.sync.dma_start(wk_sb[:], wk.rearrange("(c p) d -> p c d", p=128))
            nc.sync.dma_start(wv_sb[:], wv.rearrange("(c p) d -> p c d", p=128))

            # bv broadcast to 128 partitions (K=1 ones matmul)
            bv_ps = proj_ps.tile([128, 256], F32, tag="bv_ps")
            nc.tensor.matmul(bv_ps[:], ones1[:], bv_row[:], start=True, stop=True)
            nc.vector.tensor_copy(bv_bc[:], bv_ps[:])

            # Q/K projections: QT[hd, s] = wq_slice^T @ xT per pair
            for w_sb, dst, bcol in ((wq_sb, qt_sb, 0), (wk_sb, kt_sb, 1)):
                for p2 in range(PAIRS):
                    for g in range(SG):
                        qk_ps = proj_ps.tile([128, 512], F32, tag="qk_ps")
                        for k in range(KC):
                            nc.tensor.matmul(
                                qk_ps[:],
                                w_sb[:, k, 128 * p2 : 128 * (p2 + 1)],
                                xt_sb[:, k, 512 * g : 512 * (g + 1)],
                                start=(k == 0),
                                stop=(k == KC - 1),
                            )
                        nc.vector.tensor_scalar_add(
                            dst[:, p2, 512 * g : 512 * (g + 1)],
                            qk_ps[:],
                            bqk_sb[:, bcol, p2 : p2 + 1],
                        )

            # V projection directly in [t, hd] layout: V4 = xT^T-chunk @ wv
            for t in range(ST):
                v_ps = proj_ps.tile([128, 256], F32, tag="v_ps")
                for k in range(KC):
                    nc.tensor.matmul(
                        v_ps[:],
                        xt_sb[:, k, 128 * t : 128 * (t + 1)],
                        wv_sb[:, k, :],
                        start=(k == 0),
                        stop=(k == KC - 1),
                    )
                nc.vector.tensor_add(v4_sb[:, t, :], v_ps[:], bv_bc[:])

        for p2 in range(PAIRS):
            # ---- phase B: scores [s, t], softmax, attn output ----
            with (
                tc.tile_pool(name="sc_ps", bufs=2, space="PSUM") as sc_pool,
                tc.tile_pool(name="b_sb", bufs=3) as b_sb,
                tc.tile_pool(name="bn_sb", bufs=6) as bn_sb,
            ):
                for st in range(ST):
                    sc0 = sc_pool.tile([128, S], F32, tag="sc")
                    sc1 = sc_pool.tile([128, S], F32, tag="sc")
                    for g in range(SG):
                        for h, sc_ps in ((0, sc0), (1, sc1)):
                            nc.tensor.matmul(
                                sc_ps[:, 512 * g : 512 * (g + 1)],
                                qt_sb[64 * h : 64 * (h + 1), p2, 128 * st : 128 * (st + 1)],
                                kt_sb[64 * h : 64 * (h + 1), p2, 512 * g : 512 * (g + 1)],
                                start=True,
                                stop=True,
                                tile_position=(64 * h, 0),
                            )
                    for h, sc_ps in ((0, sc0), (1, sc1)):
                        hh = 2 * p2 + h
                        a_sb = b_sb.tile([128, S], F32, tag="a")
                        rsum = b_sb.tile([128, 1], F32, tag="rsum")
                        nc.scalar.activation(
                            a_sb[:], sc_ps[:], AF.Exp, scale=0.125, accum_out=rsum[:]
                        )
                        nc.vector.reciprocal(rrec_cols[:, hh, st : st + 1], rsum[:])
                        an_sb = bn_sb.tile([128, S], F32, tag="an")
                        nc.vector.tensor_scalar_mul(
                            an_sb[:], a_sb[:], rrec_cols[:, hh, st : st + 1]
                        )
                        nc.sync.dma_start(
                            attn[hh, 128 * st : 128 * (st + 1), :], an_sb[:]
                        )

            # denominator reciprocals, transposed to a single-partition row
            # layout (matmul rhs base partition must be 0)
            with tc.tile_pool(name="tr_ps", bufs=2, space="PSUM") as tr_pool:
                for h in range(2):
                    hh = 2 * p2 + h
                    for st in range(ST):
                        rr_ps = tr_pool.tile([1, 128], F32, tag="rr")
                        nc.tensor.transpose(
                            rr_ps[:], rrec_cols[:, hh, st : st + 1], ident[:]
                        )
                        nc.vector.tensor_copy(
                            rrec_rows[:, hh, 128 * st : 128 * (st + 1)], rr_ps[:]
                        )

            # ---- phase C: transposed scores, exp -> bf16, attn*V ----
            with (
                tc.tile_pool(name="av_ps", bufs=1, space="PSUM") as av_pool,
                tc.tile_pool(name="c_sb", bufs=3) as c_sb,
                tc.tile_pool(name="sct_ps", bufs=1, space="PSUM") as sct_pool,
            ):
                av_ps = av_pool.tile([128, S], F32, tag="av")
                for tt in range(ST):
                    for half in range(2):
                        # both heads' transposed scores for one s-half packed in
                        # one psum tile: cols [0,1024) = h0, [1024,2048) = h1
                        sct_ps = sct_pool.tile([128, S], F32, tag="sct")
                        for g in range(2):
                            for h in range(2):
                                nc.tensor.matmul(
                                    sct_ps[:, 1024 * h + 512 * g : 1024 * h + 512 * (g + 1)],
                                    kt_sb[64 * h : 64 * (h + 1), p2, 128 * tt : 128 * (tt + 1)],
                                    qt_sb[
                                        64 * h : 64 * (h + 1),
                                        p2,
                                        1024 * half + 512 * g : 1024 * half + 512 * (g + 1),
                                    ],
                                    start=True,
                                    stop=True,
                                    tile_position=(64 * h, 0),
                                )
                        at_sb = c_sb.tile([128, S], BF16, tag="at")
                        nc.scalar.activation(at_sb[:], sct_ps[:], AF.Exp, scale=0.125)
                        for g in range(2):
                            for h in range(2):
                                hh = 2 * p2 + h
                                sb = 1024 * half + 512 * g
                                nc.tensor.matmul(
                                    av_ps[64 * h : 64 * (h + 1), sb : sb + 512],
                                    v4_sb[:, tt, 64 * hh : 64 * (hh + 1)],
                                    at_sb[:, 1024 * h + 512 * g : 1024 * h + 512 * (g + 1)],
                                    start=(tt == 0),
                                    stop=(tt == ST - 1),
                                    tile_position=(0, 64 * h),
                                )

                # normalize AVT columns by the per-(s, head) reciprocal sums
                # (reuse the sct psum slot, free by now, via the same pool/tag)
                if True:
                    r2_pool = sct_pool
                    r2_ps = r2_pool.tile([128, S], F32, tag="sct")
                    for h in range(2):
                        hh = 2 * p2 + h
                        for st in range(ST):
                            nc.tensor.matmul(
                                r2_ps[:, 128 * st : 128 * (st + 1)],
                                ones1[:],
                                rrec_rows[:, hh, 128 * st : 128 * (st + 1)],
                                start=True,
                                stop=True,
                            )
                        r2_sb = c_sb.tile([128, S], F32, tag="r2sb")
                        nc.vector.tensor_copy(
                            r2_sb[64 * h : 64 * (h + 1), :],
                            r2_ps[64 * h : 64 * (h + 1), :],
                        )
                        nc.vector.tensor_mul(
                            avt_sb[64 * h : 64 * (h + 1), p2, :],
                            av_ps[64 * h : 64 * (h + 1), :],
                            r2_sb[64 * h : 64 * (h + 1), :],
                        )

        # ---- phase D: output projection, partial over this core's heads ----
        with (
            tc.tile_pool(name="o_ps", bufs=4, space="PSUM") as o_pool,
            tc.tile_pool(name="o_sb", bufs=3) as o_sb,
        ):
            for st in range(ST):
                out_sb = o_sb.tile([128, D], F32, tag="out")
                for g in range(2):
                    o_ps = o_pool.tile([128, 512], F32, tag="o")
                    for p2 in range(PAIRS):
                        nc.tensor.matmul(
                            o_ps[:],
                            avt_sb[:, p2, 128 * st : 128 * (st + 1)],
                            wo_sb[:, p2, 512 * g : 512 * (g + 1)],
                            start=(p2 == 0),
                            stop=(p2 == PAIRS - 1),
                        )
                    nc.vector.tensor_copy(out_sb[:, 512 * g : 512 * (g + 1)], o_ps[:])
                nc.sync.dma_start(outp[128 * st : 128 * (st + 1), :], out_sb[:])

    nc.compile()
    return nc


def get_nc():
    if "nc" not in _CACHE:
        _CACHE["nc"] = build()
    return _CACHE["nc"]


def kernel(x, wq, bq, wk, bk, wv, bv, wo, bo):
    x = np.asarray(x, dtype=np.float32)
    wq, bq = np.asarray(wq, np.float32), np.asarray(bq, np.float32)
    wk, bk = np.asarray(wk, np.float32), np.asarray(bk, np.float32)
    wv, bv = np.asarray(wv, np.float32), np.asarray(bv, np.float32)
    wo, bo = np.asarray(wo, np.float32), np.asarray(bo, np.float32)

    nc = get_nc()

    xts = [np.ascontiguousarray(x[b].T) for b in range(B)]
    in_maps = []
    for c in range(N_CORES):
        b, hq = c // 4, c % 4
        lo = 256 * hq
        in_maps.append(
            {
                "xt": xts[b],
                "wq": np.ascontiguousarray(wq[:, lo : lo + 256]),
                "wk": np.ascontiguousarray(wk[:, lo : lo + 256]),
                "wv": np.ascontiguousarray(wv[:, lo : lo + 256]),
                "wo": np.ascontiguousarray(wo[lo : lo + 256, :]),
                "bq": np.ascontiguousarray(bq[lo : lo + 256]),
                "bk": np.ascontiguousarray(bk[lo : lo + 256]),
                "bv": np.ascontiguousarray(bv[lo : lo + 256]),
            }
        )

    r = run_bass_kernel_spmd(nc, in_maps, list(range(N_CORES)))

    attn = np.empty((B, H, S, S), dtype=np.float32)
    out = np.zeros((B, S, D), dtype=np.float32)
    for c in range(N_CORES):
        b, hq = c // 4, c % 4
        attn[b, 4 * hq : 4 * hq + 4] = r.results[c]["attn"]
        out[b] += r.results[c]["outp"]
    out += bo
    return out, attn


# revision 13
# speedup vs baseline: 1.4808x; 1.1926x over previous
"""BERT self-attention on 8 Trainium2 NeuronCores.

Reference computes, for x [2, 2048, 1024] and 16 heads of dim 64:
    q,k,v = (x@w* + b*) split into heads
    attn  = softmax(q k^T / 8)              -> [2, 16, 2048, 2048] (output 2)
    out   = (attn v) @ wo + bo              -> [2, 2048, 1024]     (output 1)

Sharding: core c handles batch c//4 and heads 4*(c%4) .. +4 (tensor parallel
over heads, data parallel over batch). Each core computes a full partial of
the output projection over its 4 heads; host sums the 4 partials per batch
(cheaper than an on-device all-reduce) and adds bo.

Per-core kernel (all matmuls fp32r except attn*V in bf16):
  A: QT/KT [head_dim*2, s] per head-pair and V4 [t, 256] projections from a
     host-pre-transposed xT, biases fused into the PSUM->SBUF copies.
  B: per s-tile: scores for both heads of a pair concurrently (K=64
     row-packing via tile_position), ACT exp with accum_out row sums,
     DVE reciprocal + normalize, DMA of normalized attn rows.
  C: per t-tile: transposed scores, exp -> bf16, bf16 attn*V accumulation
     (M=64 col-packing); then denominators broadcast across partitions via
     K=1 ones-matmuls and DVE normalize of the [head_dim, s] result.
  D: output projection (K=256 over the two pair chunks) -> partial out.
"""

from contextlib import ExitStack

import numpy as np

import concourse.mybir as mybir
import concourse.tile as tile
from concourse import bacc
from concourse.bass_utils import run_bass_kernel_spmd
from concourse.masks import make_identity

F32 = mybir.dt.float32
F32R = mybir.dt.float32r
BF16 = mybir.dt.bfloat16
AF = mybir.ActivationFunctionType

N_CORES = 8
B = 2
S = 2048
D = 1024
H = 16
DK = 64
HEADS_PER_CORE = 4  # 2 pairs
PAIRS = 2
ST = S // 128  # 16 s/t tiles
KC = D // 128  # 8 contraction chunks
SG = S // 512  # 4 column groups of 512

_CACHE = {}


def build():
    nc = bacc.Bacc("TRN2", target_bir_lowering=False, debug=False)
    xt = nc.dram_tensor("xt", [D, S], F32R, kind="ExternalInput").ap()
    wq = nc.dram_tensor("wq", [D, 256], F32R, kind="ExternalInput").ap()
    wk = nc.dram_tensor("wk", [D, 256], F32R, kind="ExternalInput").ap()
    wv = nc.dram_tensor("wv", [D, 256], F32R, kind="ExternalInput").ap()
    wo = nc.dram_tensor("wo", [256, D], F32R, kind="ExternalInput").ap()
    bq = nc.dram_tensor("bq", [256], F32, kind="ExternalInput").ap()
    bk = nc.dram_tensor("bk", [256], F32, kind="ExternalInput").ap()
    bv = nc.dram_tensor("bv", [256], F32R, kind="ExternalInput").ap()
    attn = nc.dram_tensor("attn", [HEADS_PER_CORE, S, S], F32, kind="ExternalOutput").ap()
    outp = nc.dram_tensor("outp", [S, D], F32, kind="ExternalOutput").ap()

    with tile.TileContext(nc) as tc, ExitStack() as ctx:
        # long-lived tiles
        persist = ctx.enter_context(tc.tile_pool(name="persist", bufs=1))

        qt_sb = persist.tile([128, PAIRS, S], F32R, tag="qt")  # [hd, pair, s]
        kt_sb = persist.tile([128, PAIRS, S], F32R, tag="kt")
        v4_sb = persist.tile([128, ST, 256], BF16, tag="v4")  # [t, t_tile, hd]
        avt_sb = persist.tile([128, PAIRS, S], F32R, tag="avt")  # normalized (A@V)^T
        wo_sb = persist.tile([128, PAIRS, D], F32R, tag="wo")
        rrec_cols = persist.tile([128, HEADS_PER_CORE, ST], F32, tag="rrec_cols")
        rrec_rows = persist.tile([1, HEADS_PER_CORE, S], F32R, tag="rrec_rows")
        ones1_f32 = persist.tile([1, 128], F32, tag="ones1_f32")
        ones1 = persist.tile([1, 128], F32R, tag="ones1")
        ident = persist.tile([128, 128], F32, tag="ident")
        bqk_sb = persist.tile([128, 2, PAIRS], F32, tag="bqk")  # [p, (bq|bk), pair]
        bv_row = persist.tile([1, 256], F32R, tag="bv_row")
        bv_bc = persist.tile([128, 256], F32, tag="bv_bc")

        nc.gpsimd.memset(ones1_f32[:], 1.0)
        nc.vector.tensor_copy(ones1[:], ones1_f32[:])
        make_identity(nc, ident[:])
        # matmul inputs are declared float32r in DRAM (same bits as f32), so
        # plain HWDGE DMA satisfies the verifier's fp32r producer rule
        nc.sync.dma_start(wo_sb[:], wo.rearrange("(c p) d -> p c d", p=128))
        nc.sync.dma_start(bqk_sb[:, 0, :], bq.rearrange("(c p) -> p c", p=128))
        nc.sync.dma_start(bqk_sb[:, 1, :], bk.rearrange("(c p) -> p c", p=128))
        nc.sync.dma_start(bv_row[:], bv.rearrange("(o d) -> o d", o=1))

        with (
            tc.tile_pool(name="proj_in", bufs=1) as proj_in,
            tc.tile_pool(name="proj_ps", bufs=2, space="PSUM") as proj_ps,
        ):
            xt_sb = proj_in.tile([128, KC, S], F32R, tag="xt")
            wq_sb = proj_in.tile([128, KC, 256], F32R, tag="wq")
            wk_sb = proj_in.tile([128, KC, 256], F32R, tag="wk")
            wv_sb = proj_in.tile([128, KC, 256], F32R, tag="wv")
            nc.sync.dma_start(wq_sb[:], wq.rearrange("(c p) d -> p c d", p=128))
            nc.sync.dma_start(wk_sb[:], wk.rearrange("(c p) d -> p c d", p=128))
            nc.sync.dma_start(wv_sb[:], wv.rearrange("(c p) d -> p c d", p=128))
            xt_r = xt.rearrange("(c p) s -> p c s", p=128)
            for k in range(KC):
                nc.sync.dma_start(xt_sb[:, k, :], xt_r[:, k, :])

            # bv broadcast to 128 partitions (K=1 ones matmul)
            bv_ps = proj_ps.tile([128, 256], F32, tag="bv_ps")
            nc.tensor.matmul(bv_ps[:], ones1[:], bv_row[:], start=True, stop=True)
            nc.vector.tensor_copy(bv_bc[:], bv_ps[:])

            # Q/K projections: QT[hd, s] = wq_slice^T @ xT per pair
            for w_sb, dst, bcol in ((wq_sb, qt_sb, 0), (wk_sb, kt_sb, 1)):
                for p2 in range(PAIRS):
                    for g in range(SG):
                        qk_ps = proj_ps.tile([128, 512], F32, tag="qk_ps")
                        for k in range(KC):
                            nc.tensor.matmul(
                                qk_ps[:],
                                w_sb[:, k, 128 * p2 : 128 * (p2 + 1)],
                                xt_sb[:, k, 512 * g : 512 * (g + 1)],
                                start=(k == 0),
                                stop=(k == KC - 1),
                            )
                        nc.vector.tensor_scalar_add(
                            dst[:, p2, 512 * g : 512 * (g + 1)],
                            qk_ps[:],
                            bqk_sb[:, bcol, p2 : p2 + 1],
                        )

            # V projection directly in [t, hd] layout: V4 = xT^T-chunk @ wv
            for t in range(ST):
                v_ps = proj_ps.tile([128, 256], F32, tag="v_ps")
                for k in range(KC):
                    nc.tensor.matmul(
                        v_ps[:],
                        xt_sb[:, k, 128 * t : 128 * (t + 1)],
                        wv_sb[:, k, :],
                        start=(k == 0),
                        stop=(k == KC - 1),
                    )
                nc.vector.tensor_add(v4_sb[:, t, :], v_ps[:], bv_bc[:])

        for p2 in range(PAIRS):
            # ---- phase B: scores [s, t], softmax, attn output ----
            with (
                tc.tile_pool(name="sc_ps", bufs=2, space="PSUM") as sc_pool,
                tc.tile_pool(name="b_sb", bufs=3) as b_sb,
                tc.tile_pool(name="bn_sb", bufs=6) as bn_sb,
            ):
                for st in range(ST):
                    sc0 = sc_pool.tile([128, S], F32, tag="sc")
                    sc1 = sc_pool.tile([128, S], F32, tag="sc")
                    for g in range(SG):
                        for h, sc_ps in ((0, sc0), (1, sc1)):
                            nc.tensor.matmul(
                                sc_ps[:, 512 * g : 512 * (g + 1)],
                                qt_sb[64 * h : 64 * (h + 1), p2, 128 * st : 128 * (st + 1)],
                                kt_sb[64 * h : 64 * (h + 1), p2, 512 * g : 512 * (g + 1)],
                                start=True,
                                stop=True,
                                tile_position=(64 * h, 0),
                            )
                    for h, sc_ps in ((0, sc0), (1, sc1)):
                        hh = 2 * p2 + h
                        a_sb = b_sb.tile([128, S], F32, tag="a")
                        rsum = b_sb.tile([128, 1], F32, tag="rsum")
                        nc.scalar.activation(
                            a_sb[:], sc_ps[:], AF.Exp, scale=0.125, accum_out=rsum[:]
                        )
                        nc.vector.reciprocal(rrec_cols[:, hh, st : st + 1], rsum[:])
                        an_sb = bn_sb.tile([128, S], F32, tag="an")
                        nc.vector.tensor_scalar_mul(
                            an_sb[:], a_sb[:], rrec_cols[:, hh, st : st + 1]
                        )
                        nc.sync.dma_start(
                            attn[hh, 128 * st : 128 * (st + 1), :], an_sb[:]
                        )

            # denominator reciprocals, transposed to a single-partition row
            # layout (matmul rhs base partition must be 0)
            with tc.tile_pool(name="tr_ps", bufs=2, space="PSUM") as tr_pool:
                for h in range(2):
                    hh = 2 * p2 + h
                    for st in range(ST):
                        rr_ps = tr_pool.tile([1, 128], F32, tag="rr")
                        nc.tensor.transpose(
                            rr_ps[:], rrec_cols[:, hh, st : st + 1], ident[:]
                        )
                        nc.vector.tensor_copy(
                            rrec_rows[:, hh, 128 * st : 128 * (st + 1)], rr_ps[:]
                        )

            # ---- phase C: transposed scores, exp -> bf16, attn*V ----
            with (
                tc.tile_pool(name="av_ps", bufs=1, space="PSUM") as av_pool,
                tc.tile_pool(name="c_sb", bufs=4) as c_sb,
                tc.tile_pool(name="sct_ps", bufs=2, space="PSUM") as sct_pool,
                tc.tile_pool(name="warm_ps", bufs=1, space="PSUM") as warm_pool,
            ):
                # dense matmul streak to flip the PE HAM clock-gate to 8/8
                warm_ps = warm_pool.tile([128, 512], F32, tag="warm")
                for _ in range(12):
                    nc.tensor.matmul(
                        warm_ps[:],
                        qt_sb[:, p2, 0:128],
                        kt_sb[:, p2, 0:512],
                        start=True,
                        stop=True,
                    )

                for half in range(2):
                    av_ps = av_pool.tile([128, 1024], F32, tag="av")
                    for tt in range(ST):
                        for q in range(2):
                            # one s-quarter (512 cols) for both heads:
                            # cols [0,512) = h0, [512,1024) = h1
                            sct_ps = sct_pool.tile([128, 1024], F32, tag="sct")
                            sq = 1024 * half + 512 * q
                            for h in range(2):
                                nc.tensor.matmul(
                                    sct_ps[:, 512 * h : 512 * (h + 1)],
                                    kt_sb[64 * h : 64 * (h + 1), p2, 128 * tt : 128 * (tt + 1)],
                                    qt_sb[64 * h : 64 * (h + 1), p2, sq : sq + 512],
                                    start=True,
                                    stop=True,
                                    tile_position=(64 * h, 0),
                                )
                            at_sb = c_sb.tile([128, 1024], BF16, tag="at")
                            nc.scalar.activation(
                                at_sb[:], sct_ps[:], AF.Exp, scale=0.125
                            )
                            for h in range(2):
                                hh = 2 * p2 + h
                                nc.tensor.matmul(
                                    av_ps[64 * h : 64 * (h + 1), 512 * q : 512 * (q + 1)],
                                    v4_sb[:, tt, 64 * hh : 64 * (hh + 1)],
                                    at_sb[:, 512 * h : 512 * (h + 1)],
                                    start=(tt == 0),
                                    stop=(tt == ST - 1),
                                    tile_position=(0, 64 * h),
                                )

                    # normalize this half's AVT columns by the reciprocal sums
                    r2_ps = sct_pool.tile([128, 1024], F32, tag="sct")
                    for h in range(2):
                        hh = 2 * p2 + h
                        for st2 in range(8):
                            st = 8 * half + st2
                            nc.tensor.matmul(
                                r2_ps[:, 128 * st2 : 128 * (st2 + 1)],
                                ones1[:],
                                rrec_rows[:, hh, 128 * st : 128 * (st + 1)],
                                start=True,
                                stop=True,
                            )
                        r2_sb = c_sb.tile([128, 1024], F32, tag="r2sb")
                        nc.vector.tensor_copy(
                            r2_sb[64 * h : 64 * (h + 1), :],
                            r2_ps[64 * h : 64 * (h + 1), :],
                        )
                        nc.vector.tensor_mul(
                            avt_sb[64 * h : 64 * (h + 1), p2, 1024 * half : 1024 * (half + 1)],
                            av_ps[64 * h : 64 * (h + 1), :],
                            r2_sb[64 * h : 64 * (h + 1), :],
                        )

        # ---- phase D: output projection, partial over this core's heads ----
        with (
            tc.tile_pool(name="o_ps", bufs=4, space="PSUM") as o_pool,
            tc.tile_pool(name="o_sb", bufs=3) as o_sb,
        ):
            for st in range(ST):
                out_sb = o_sb.tile([128, D], F32, tag="out")
                for g in range(2):
                    o_ps = o_pool.tile([128, 512], F32, tag="o")
                    for p2 in range(PAIRS):
                        nc.tensor.matmul(
                            o_ps[:],
                            avt_sb[:, p2, 128 * st : 128 * (st + 1)],
                            wo_sb[:, p2, 512 * g : 512 * (g + 1)],
                            start=(p2 == 0),
                            stop=(p2 == PAIRS - 1),
                        )
                    nc.vector.tensor_copy(out_sb[:, 512 * g : 512 * (g + 1)], o_ps[:])
                nc.sync.dma_start(outp[128 * st : 128 * (st + 1), :], out_sb[:])

    nc.compile()
    return nc


def get_nc():
    if "nc" not in _CACHE:
        _CACHE["nc"] = build()
    return _CACHE["nc"]


def kernel(x, wq, bq, wk, bk, wv, bv, wo, bo):
    x = np.asarray(x, dtype=np.float32)
    wq, bq = np.asarray(wq, np.float32), np.asarray(bq, np.float32)
    wk, bk = np.asarray(wk, np.float32), np.asarray(bk, np.float32)
    wv, bv = np.asarray(wv, np.float32), np.asarray(bv, np.float32)
    wo, bo = np.asarray(wo, np.float32), np.asarray(bo, np.float32)

    nc = get_nc()

    xts = [np.ascontiguousarray(x[b].T) for b in range(B)]
    in_maps = []
    for c in range(N_CORES):
        b, hq = c // 4, c % 4
        lo = 256 * hq
        in_maps.append(
            {
                "xt": xts[b],
                "wq": np.ascontiguousarray(wq[:, lo : lo + 256]),
                "wk": np.ascontiguousarray(wk[:, lo : lo + 256]),
                "wv": np.ascontiguousarray(wv[:, lo : lo + 256]),
                "wo": np.ascontiguousarray(wo[lo : lo + 256, :]),
                "bq": np.ascontiguousarray(bq[lo : lo + 256]),
                "bk": np.ascontiguousarray(bk[lo : lo + 256]),
                "bv": np.ascontiguousarray(bv[lo : lo + 256]),
            }
        )

    r = run_bass_kernel_spmd(nc, in_maps, list(range(N_CORES)))

    attn = np.empty((B, H, S, S), dtype=np.float32)
    out = np.zeros((B, S, D), dtype=np.float32)
    for c in range(N_CORES):
        b, hq = c // 4, c % 4
        attn[b, 4 * hq : 4 * hq + 4] = r.results[c]["attn"]
        out[b] += r.results[c]["outp"]
    out += bo
    return out, attn


# revision 14
# speedup vs baseline: 1.6128x; 1.0891x over previous
"""BERT self-attention on 8 Trainium2 NeuronCores.

Reference computes, for x [2, 2048, 1024] and 16 heads of dim 64:
    q,k,v = (x@w* + b*) split into heads
    attn  = softmax(q k^T / 8)              -> [2, 16, 2048, 2048] (output 2)
    out   = (attn v) @ wo + bo              -> [2, 2048, 1024]     (output 1)

Sharding: core c handles batch c//4 and heads 4*(c%4) .. +4 (tensor parallel
over heads, data parallel over batch). Each core computes a full partial of
the output projection over its 4 heads; host sums the 4 partials per batch
(cheaper than an on-device all-reduce) and adds bo.

Per-core kernel (all matmuls fp32r except attn*V in bf16):
  A: QT/KT [head_dim*2, s] per head-pair and V4 [t, 256] projections from a
     host-pre-transposed xT, biases fused into the PSUM->SBUF copies.
  B: per s-tile: scores for both heads of a pair concurrently (K=64
     row-packing via tile_position), ACT exp with accum_out row sums,
     DVE reciprocal + normalize, DMA of normalized attn rows.
  C: per t-tile: transposed scores, exp -> bf16, bf16 attn*V accumulation
     (M=64 col-packing); then denominators broadcast across partitions via
     K=1 ones-matmuls and DVE normalize of the [head_dim, s] result.
  D: output projection (K=256 over the two pair chunks) -> partial out.
"""

from contextlib import ExitStack

import numpy as np

import concourse.mybir as mybir
import concourse.tile as tile
from concourse import bacc
from concourse.bass_utils import run_bass_kernel_spmd
from concourse.masks import make_identity

F32 = mybir.dt.float32
F32R = mybir.dt.float32r
BF16 = mybir.dt.bfloat16
AF = mybir.ActivationFunctionType

N_CORES = 8
B = 2
S = 2048
D = 1024
H = 16
DK = 64
HEADS_PER_CORE = 4  # 2 pairs
PAIRS = 2
ST = S // 128  # 16 s/t tiles
KC = D // 128  # 8 contraction chunks
SG = S // 512  # 4 column groups of 512

_CACHE = {}


def build():
    nc = bacc.Bacc("TRN2", target_bir_lowering=False, debug=False)
    xt = nc.dram_tensor("xt", [D, S], F32R, kind="ExternalInput").ap()
    wq = nc.dram_tensor("wq", [D, 256], F32R, kind="ExternalInput").ap()
    wk = nc.dram_tensor("wk", [D, 256], F32R, kind="ExternalInput").ap()
    wv = nc.dram_tensor("wv", [D, 256], F32R, kind="ExternalInput").ap()
    wo = nc.dram_tensor("wo", [256, D], F32R, kind="ExternalInput").ap()
    bq = nc.dram_tensor("bq", [256], F32, kind="ExternalInput").ap()
    bk = nc.dram_tensor("bk", [256], F32, kind="ExternalInput").ap()
    bv = nc.dram_tensor("bv", [256], F32R, kind="ExternalInput").ap()
    attn = nc.dram_tensor("attn", [HEADS_PER_CORE, S, S], F32, kind="ExternalOutput").ap()
    outp = nc.dram_tensor("outp", [S, D], F32, kind="ExternalOutput").ap()

    with tile.TileContext(nc) as tc, ExitStack() as ctx:
        # long-lived tiles
        persist = ctx.enter_context(tc.tile_pool(name="persist", bufs=1))

        qt_sb = persist.tile([128, PAIRS, S], F32R, tag="qt")  # [hd, pair, s]
        kt_sb = persist.tile([128, PAIRS, S], F32R, tag="kt")
        v4_sb = persist.tile([128, ST, 256], BF16, tag="v4")  # [t, t_tile, hd]
        avt_sb = persist.tile([128, PAIRS, S], F32R, tag="avt")  # normalized (A@V)^T
        wo_sb = persist.tile([128, PAIRS, D], F32R, tag="wo")
        rrec_cols = persist.tile([128, HEADS_PER_CORE, ST], F32, tag="rrec_cols")
        rrec_rows = persist.tile([1, HEADS_PER_CORE, S], F32R, tag="rrec_rows")
        ones1_f32 = persist.tile([1, 128], F32, tag="ones1_f32")
        ones1 = persist.tile([1, 128], F32R, tag="ones1")
        ident = persist.tile([128, 128], F32, tag="ident")
        bqk_sb = persist.tile([128, 2, PAIRS], F32, tag="bqk")  # [p, (bq|bk), pair]
        bv_row = persist.tile([1, 256], F32R, tag="bv_row")
        bv_bc = persist.tile([128, 256], F32, tag="bv_bc")

        nc.gpsimd.memset(ones1_f32[:], 1.0)
        nc.vector.tensor_copy(ones1[:], ones1_f32[:])
        make_identity(nc, ident[:])
        # matmul inputs are declared float32r in DRAM (same bits as f32), so
        # plain HWDGE DMA satisfies the verifier's fp32r producer rule
        nc.sync.dma_start(wo_sb[:], wo.rearrange("(c p) d -> p c d", p=128))
        nc.sync.dma_start(bqk_sb[:, 0, :], bq.rearrange("(c p) -> p c", p=128))
        nc.sync.dma_start(bqk_sb[:, 1, :], bk.rearrange("(c p) -> p c", p=128))
        nc.sync.dma_start(bv_row[:], bv.rearrange("(o d) -> o d", o=1))

        with (
            tc.tile_pool(name="proj_in", bufs=1) as proj_in,
            tc.tile_pool(name="proj_ps", bufs=2, space="PSUM") as proj_ps,
        ):
            xt_sb = proj_in.tile([128, KC, S], F32R, tag="xt")
            wq_sb = proj_in.tile([128, KC, 256], F32R, tag="wq")
            wk_sb = proj_in.tile([128, KC, 256], F32R, tag="wk")
            wv_sb = proj_in.tile([128, KC, 256], F32R, tag="wv")
            nc.sync.dma_start(wq_sb[:], wq.rearrange("(c p) d -> p c d", p=128))
            nc.sync.dma_start(wk_sb[:], wk.rearrange("(c p) d -> p c d", p=128))
            nc.sync.dma_start(wv_sb[:], wv.rearrange("(c p) d -> p c d", p=128))
            xt_r = xt.rearrange("(c p) s -> p c s", p=128)
            for k in range(KC):
                nc.sync.dma_start(xt_sb[:, k, :], xt_r[:, k, :])

            # bv broadcast to 128 partitions (K=1 ones matmul)
            bv_ps = proj_ps.tile([128, 256], F32, tag="bv_ps")
            nc.tensor.matmul(bv_ps[:], ones1[:], bv_row[:], start=True, stop=True)
            nc.vector.tensor_copy(bv_bc[:], bv_ps[:])

            # Q/K projections: QT[hd, s] = wq_slice^T @ xT per pair
            for w_sb, dst, bcol in ((wq_sb, qt_sb, 0), (wk_sb, kt_sb, 1)):
                for p2 in range(PAIRS):
                    for g in range(SG):
                        qk_ps = proj_ps.tile([128, 512], F32, tag="qk_ps")
                        for k in range(KC):
                            nc.tensor.matmul(
                                qk_ps[:],
                                w_sb[:, k, 128 * p2 : 128 * (p2 + 1)],
                                xt_sb[:, k, 512 * g : 512 * (g + 1)],
                                start=(k == 0),
                                stop=(k == KC - 1),
                            )
                        nc.vector.tensor_scalar_add(
                            dst[:, p2, 512 * g : 512 * (g + 1)],
                            qk_ps[:],
                            bqk_sb[:, bcol, p2 : p2 + 1],
                        )

            # V projection directly in [t, hd] layout: V4 = xT^T-chunk @ wv
            for t in range(ST):
                v_ps = proj_ps.tile([128, 256], F32, tag="v_ps")
                for k in range(KC):
                    nc.tensor.matmul(
                        v_ps[:],
                        xt_sb[:, k, 128 * t : 128 * (t + 1)],
                        wv_sb[:, k, :],
                        start=(k == 0),
                        stop=(k == KC - 1),
                    )
                nc.vector.tensor_add(v4_sb[:, t, :], v_ps[:], bv_bc[:])

        for p2 in range(PAIRS):
            # ---- phase B: scores [s, t], softmax, attn output ----
            with (
                tc.tile_pool(name="sc_ps", bufs=2, space="PSUM") as sc_pool,
                tc.tile_pool(name="b_sb", bufs=3) as b_sb,
                tc.tile_pool(name="bn_sb", bufs=6) as bn_sb,
            ):
                for st in range(ST):
                    sc0 = sc_pool.tile([128, S], F32, tag="sc")
                    sc1 = sc_pool.tile([128, S], F32, tag="sc")
                    for g in range(SG):
                        for h, sc_ps in ((0, sc0), (1, sc1)):
                            nc.tensor.matmul(
                                sc_ps[:, 512 * g : 512 * (g + 1)],
                                qt_sb[64 * h : 64 * (h + 1), p2, 128 * st : 128 * (st + 1)],
                                kt_sb[64 * h : 64 * (h + 1), p2, 512 * g : 512 * (g + 1)],
                                start=True,
                                stop=True,
                                tile_position=(64 * h, 0),
                            )
                    for h, sc_ps in ((0, sc0), (1, sc1)):
                        hh = 2 * p2 + h
                        a_sb = b_sb.tile([128, S], F32, tag="a")
                        rsum = b_sb.tile([128, 1], F32, tag="rsum")
                        nc.scalar.activation(
                            a_sb[:], sc_ps[:], AF.Exp, scale=0.125, accum_out=rsum[:]
                        )
                        nc.vector.reciprocal(rrec_cols[:, hh, st : st + 1], rsum[:])
                        an_sb = bn_sb.tile([128, S], F32, tag="an")
                        nc.vector.tensor_scalar_mul(
                            an_sb[:], a_sb[:], rrec_cols[:, hh, st : st + 1]
                        )
                        nc.sync.dma_start(
                            attn[hh, 128 * st : 128 * (st + 1), :], an_sb[:]
                        )

            # denominator reciprocals, transposed to a single-partition row
            # layout (matmul rhs base partition must be 0)
            with tc.tile_pool(name="tr_ps", bufs=2, space="PSUM") as tr_pool:
                for h in range(2):
                    hh = 2 * p2 + h
                    for st in range(ST):
                        rr_ps = tr_pool.tile([1, 128], F32, tag="rr")
                        nc.tensor.transpose(
                            rr_ps[:], rrec_cols[:, hh, st : st + 1], ident[:]
                        )
                        nc.vector.tensor_copy(
                            rrec_rows[:, hh, 128 * st : 128 * (st + 1)], rr_ps[:]
                        )

            # ---- phase C: transposed scores, exp -> bf16, attn*V ----
            with (
                tc.tile_pool(name="av_ps", bufs=1, space="PSUM") as av_pool,
                tc.tile_pool(name="c_sb", bufs=4) as c_sb,
                tc.tile_pool(name="sct_ps", bufs=2, space="PSUM") as sct_pool,
                tc.tile_pool(name="warm_ps", bufs=1, space="PSUM") as warm_pool,
            ):
                # dense matmul streak to flip the PE HAM clock-gate to 8/8
                warm_ps = warm_pool.tile([128, 512], F32, tag="warm")
                for _ in range(12):
                    nc.tensor.matmul(
                        warm_ps[:],
                        qt_sb[:, p2, 0:128],
                        kt_sb[:, p2, 0:512],
                        start=True,
                        stop=True,
                    )

                for half in range(2):
                    av_ps = av_pool.tile([128, 1024], F32, tag="av")
                    for tt in range(ST):
                        for q in range(2):
                            # one s-quarter (512 cols) for both heads:
                            # cols [0,512) = h0, [512,1024) = h1
                            sct_ps = sct_pool.tile([128, 1024], F32, tag="sct")
                            # filler matmul: keeps the PE activity monitor busy
                            # during exp waits so the clock gate stays at 8/8
                            nc.tensor.matmul(
                                warm_ps[:],
                                qt_sb[:, p2, 0:128],
                                kt_sb[:, p2, 0:512],
                                start=True,
                                stop=True,
                            )
                            sq = 1024 * half + 512 * q
                            for h in range(2):
                                nc.tensor.matmul(
                                    sct_ps[:, 512 * h : 512 * (h + 1)],
                                    kt_sb[64 * h : 64 * (h + 1), p2, 128 * tt : 128 * (tt + 1)],
                                    qt_sb[64 * h : 64 * (h + 1), p2, sq : sq + 512],
                                    start=True,
                                    stop=True,
                                    tile_position=(64 * h, 0),
                                )
                            at_sb = c_sb.tile([128, 1024], BF16, tag="at")
                            nc.scalar.activation(
                                at_sb[:], sct_ps[:], AF.Exp, scale=0.125
                            )
                            for h in range(2):
                                hh = 2 * p2 + h
                                nc.tensor.matmul(
                                    av_ps[64 * h : 64 * (h + 1), 512 * q : 512 * (q + 1)],
                                    v4_sb[:, tt, 64 * hh : 64 * (hh + 1)],
                                    at_sb[:, 512 * h : 512 * (h + 1)],
                                    start=(tt == 0),
                                    stop=(tt == ST - 1),
                                    tile_position=(0, 64 * h),
                                )

                    # normalize this half's AVT columns by the reciprocal sums
                    r2_ps = sct_pool.tile([128, 1024], F32, tag="sct")
                    for h in range(2):
                        hh = 2 * p2 + h
                        for st2 in range(8):
                            st = 8 * half + st2
                            nc.tensor.matmul(
                                r2_ps[:, 128 * st2 : 128 * (st2 + 1)],
                                ones1[:],
                                rrec_rows[:, hh, 128 * st : 128 * (st + 1)],
                                start=True,
                                stop=True,
                            )
                        r2_sb = c_sb.tile([128, 1024], F32, tag="r2sb")
                        nc.vector.tensor_copy(
                            r2_sb[64 * h : 64 * (h + 1), :],
                            r2_ps[64 * h : 64 * (h + 1), :],
                        )
                        nc.vector.tensor_mul(
                            avt_sb[64 * h : 64 * (h + 1), p2, 1024 * half : 1024 * (half + 1)],
                            av_ps[64 * h : 64 * (h + 1), :],
                            r2_sb[64 * h : 64 * (h + 1), :],
                        )

        # ---- phase D: output projection, partial over this core's heads ----
        with (
            tc.tile_pool(name="o_ps", bufs=4, space="PSUM") as o_pool,
            tc.tile_pool(name="o_sb", bufs=3) as o_sb,
        ):
            for st in range(ST):
                out_sb = o_sb.tile([128, D], F32, tag="out")
                for g in range(2):
                    o_ps = o_pool.tile([128, 512], F32, tag="o")
                    for p2 in range(PAIRS):
                        nc.tensor.matmul(
                            o_ps[:],
                            avt_sb[:, p2, 128 * st : 128 * (st + 1)],
                            wo_sb[:, p2, 512 * g : 512 * (g + 1)],
                            start=(p2 == 0),
                            stop=(p2 == PAIRS - 1),
                        )
                    nc.vector.tensor_copy(out_sb[:, 512 * g : 512 * (g + 1)], o_ps[:])
                nc.sync.dma_start(outp[128 * st : 128 * (st + 1), :], out_sb[:])

    nc.compile()
    return nc


def get_nc():
    if "nc" not in _CACHE:
        _CACHE["nc"] = build()
    return _CACHE["nc"]


def kernel(x, wq, bq, wk, bk, wv, bv, wo, bo):
    x = np.asarray(x, dtype=np.float32)
    wq, bq = np.asarray(wq, np.float32), np.asarray(bq, np.float32)
    wk, bk = np.asarray(wk, np.float32), np.asarray(bk, np.float32)
    wv, bv = np.asarray(wv, np.float32), np.asarray(bv, np.float32)
    wo, bo = np.asarray(wo, np.float32), np.asarray(bo, np.float32)

    nc = get_nc()

    xts = [np.ascontiguousarray(x[b].T) for b in range(B)]
    in_maps = []
    for c in range(N_CORES):
        b, hq = c // 4, c % 4
        lo = 256 * hq
        in_maps.append(
            {
                "xt": xts[b],
                "wq": np.ascontiguousarray(wq[:, lo : lo + 256]),
                "wk": np.ascontiguousarray(wk[:, lo : lo + 256]),
                "wv": np.ascontiguousarray(wv[:, lo : lo + 256]),
                "wo": np.ascontiguousarray(wo[lo : lo + 256, :]),
                "bq": np.ascontiguousarray(bq[lo : lo + 256]),
                "bk": np.ascontiguousarray(bk[lo : lo + 256]),
                "bv": np.ascontiguousarray(bv[lo : lo + 256]),
            }
        )

    r = run_bass_kernel_spmd(nc, in_maps, list(range(N_CORES)))

    attn = np.empty((B, H, S, S), dtype=np.float32)
    out = np.zeros((B, S, D), dtype=np.float32)
    for c in range(N_CORES):
        b, hq = c // 4, c % 4
        attn[b, 4 * hq : 4 * hq + 4] = r.results[c]["attn"]
        out[b] += r.results[c]["outp"]
    out += bo
    return out, attn
